# revision 6
# baseline (speedup 1.0000x reference)
"""Trainium2 Bass kernel for nn_DeltaRuleMemory (decayed causal linear attention
with RoPE, ternary-STE k/v quantization and beta key gating).

Sharding: 8 cores = batch (2) x head-groups (4 groups of 4 heads). Each core
computes its (b, head-group) slice end-to-end; the only cross-core exchange is
a 2-float AllReduce for the global ternary-quantization thresholds. Host sums
the 4 per-head-group partial output projections per batch.

Algorithm note: decay alpha = sigmoid(alpha_log) < 0.5 for every head, so
exp(log_alpha * d) underflows to exactly 0.0f for d >= 128 (as it does in the
reference's T x T decay matrix). Attention is therefore computed exactly as a
banded product: each 128-query chunk attends to its own chunk (masked decay)
and the previous chunk only.

Precision: the ternary threshold compare is sensitive to k/v values, so the
k/v projections run as a 3-chain fp16 hi/lo split (x = xh + xl, W = Wh + Wl;
k = xh@Wh + xl@Wh + xh@Wl, fp32 PSUM accumulation) which is fp32-accurate to
~2^-21 but runs at bf16 matmul speed. q/beta run a single fp16 hi chain and
the attention + output projection run in fp16 (~1e-4 relative, well below the
quantization-boundary noise).
"""
import numpy as np
from contextlib import ExitStack

import concourse.bass as bass
import concourse.tile as tile
import concourse.mybir as mybir
from concourse import bacc
from concourse.bass import ds
from concourse.bass_utils import run_bass_kernel_spmd

F32 = mybir.dt.float32
F16 = mybir.dt.float16
MUL = mybir.AluOpType.mult
ADD = mybir.AluOpType.add
SUB = mybir.AluOpType.subtract

B, D_MODEL, NH, HD = 2, 1024, 16, 64
INNER = NH * HD
N_CORES = 8
HG = 4              # heads per core
GD = HG * HD        # inner dims per core (256)
C = 128             # attention chunk
ROPE_BASE = 10000.0
THR_MIN, THR_MAX = 0.01, 10.0

_NC_CACHE = {}


def build_nc(T=2048, n_cores=N_CORES, use_cc=True, repeat=1):
    """Build the SPMD bass program (identical on every core)."""
    KT = D_MODEL // 128          # 8 contraction tiles
    NCH = T // C                 # chunks
    W5 = min(512, T)             # free-dim window for [*, T] processing
    NW = T // W5

    nc = bacc.Bacc("TRN2", target_bir_lowering=False, debug=False,
                   enable_asserts=True, num_devices=n_cores)

    xh_d = nc.dram_tensor("xh", [D_MODEL, T], F16, kind="ExternalInput").ap()
    xl_d = nc.dram_tensor("xl", [D_MODEL, T], F16, kind="ExternalInput").ap()
    wkh_d = nc.dram_tensor("wkh", [D_MODEL, GD], F16, kind="ExternalInput").ap()
    wkl_d = nc.dram_tensor("wkl", [D_MODEL, GD], F16, kind="ExternalInput").ap()
    wvh_d = nc.dram_tensor("wvh", [D_MODEL, GD], F16, kind="ExternalInput").ap()
    wvl_d = nc.dram_tensor("wvl", [D_MODEL, GD], F16, kind="ExternalInput").ap()
    wq_d = nc.dram_tensor("wq", [D_MODEL, GD], F16, kind="ExternalInput").ap()
    wb_d = nc.dram_tensor("wb", [D_MODEL, HG], F16, kind="ExternalInput").ap()
    bbx_d = nc.dram_tensor("bbx", [128, HG], F32, kind="ExternalInput").ap()
    wo_d = nc.dram_tensor("wo", [GD, D_MODEL], F16, kind="ExternalInput").ap()
    ct_d = nc.dram_tensor("ct", [128, T], F32, kind="ExternalInput").ap()
    st_d = nc.dram_tensor("st", [128, T], F32, kind="ExternalInput").ap()
    dt2_d = nc.dram_tensor("dt2", [128, HG, 2 * C], F32, kind="ExternalInput").ap()
    out_d = nc.dram_tensor("out", [T, D_MODEL], F32, kind="ExternalOutput").ap()

    if use_cc:
        cc_in = nc.dram_tensor("cc_in", [1, 2], F32)
        cc_out = nc.dram_tensor("cc_out", [1, 2], F32, addr_space="Shared")

    with tile.TileContext(nc) as tc, ExitStack() as ctx:
        cpool = ctx.enter_context(tc.tile_pool(name="const", bufs=1))
        wpool = ctx.enter_context(tc.tile_pool(name="w", bufs=2))
        big = ctx.enter_context(tc.tile_pool(name="big", bufs=1))
        scr = ctx.enter_context(tc.tile_pool(name="scr", bufs=3))
        gl = ctx.enter_context(tc.tile_pool(name="gl", bufs=2))
        stp = ctx.enter_context(tc.tile_pool(name="stp", bufs=2))
        tiny = ctx.enter_context(tc.tile_pool(name="tiny", bufs=1))
        xpool = ctx.enter_context(tc.tile_pool(name="xs", bufs=2))

        # ---- weights for the k chain + first x window go first so the PE
        # can start ~4us in; everything else queues behind them ----
        wkh_sb = cpool.tile([128, KT, GD], F16, tag="wkh")
        nc.sync.dma_start(wkh_sb[:], wkh_d.rearrange("(ko p) m -> p ko m", p=128))

        def load_x(w, name_sfx):
            win = ds(w * W5, W5)
            xhw = xpool.tile([128, KT, W5], F16, tag="xh", name=f"xh{name_sfx}")
            xlw = xpool.tile([128, KT, W5], F16, tag="xl", name=f"xl{name_sfx}")
            for kt_i in range(KT):
                nc.sync.dma_start(xhw[:, kt_i, :], xh_d[ds(kt_i * 128, 128), win])
            for kt_i in range(KT):
                nc.sync.dma_start(xlw[:, kt_i, :], xl_d[ds(kt_i * 128, 128), win])
            return xhw, xlw

        x0 = load_x(0, "_0")

        wkl_sb = cpool.tile([128, KT, GD], F16, tag="wkl")
        nc.sync.dma_start(wkl_sb[:], wkl_d.rearrange("(ko p) m -> p ko m", p=128))
        wq_sb = cpool.tile([128, KT, GD], F16, tag="wq")
        nc.sync.dma_start(wq_sb[:], wq_d.rearrange("(ko p) m -> p ko m", p=128))
        ct_sb = cpool.tile([128, T], F32, tag="ct")
        st_sb = cpool.tile([128, T], F32, tag="st")
        nc.sync.dma_start(ct_sb[:], ct_d[:])
        nc.sync.dma_start(st_sb[:], st_d[:])
        wvh_sb = cpool.tile([128, KT, GD], F16, tag="wvh")
        nc.sync.dma_start(wvh_sb[:], wvh_d.rearrange("(ko p) m -> p ko m", p=128))
        wvl_sb = cpool.tile([128, KT, GD], F16, tag="wvl")
        nc.sync.dma_start(wvl_sb[:], wvl_d.rearrange("(ko p) m -> p ko m", p=128))
        wb_sb = cpool.tile([128, KT, HG], F16, tag="wb")
        nc.sync.dma_start(wb_sb[:], wb_d.rearrange("(ko p) h -> p ko h", p=128))
        bbx_sb = cpool.tile([128, HG], F32, tag="bbx")
        nc.sync.dma_start(bbx_sb[:], bbx_d[:])
        dt2_sb = cpool.tile([128, HG, 2 * C], F32, tag="dt2")
        nc.sync.dma_start(dt2_sb[:], dt2_d[:])

        for rep in range(repeat):
            sfx = f"_r{rep}" if repeat > 1 else ""

            # per-iteration persistent tensors
            kT = [big.tile([128, T], F32, tag=f"kT{i}", name=f"kT{i}{sfx}") for i in range(2)]
            kTt = [big.tile([128, T], F16, tag=f"kTt{i}", name=f"kTt{i}{sfx}") for i in range(2)]
            qT = [big.tile([128, T], F16, tag=f"qT{i}", name=f"qT{i}{sfx}") for i in range(2)]
            v_sb = big.tile([128, NCH, GD], F32, tag="v", name=f"v{sfx}")
            vbt = big.tile([128, NCH, GD], F16, tag="vbt", name=f"vbt{sfx}")
            blog = big.tile([128, NCH, HG], F32, tag="blog", name=f"blog{sfx}")
            beta = big.tile([128, NCH, HG], F32, tag="beta", name=f"beta{sfx}")
            oT = [big.tile([128, T], F16, tag=f"oT{i}", name=f"oT{i}{sfx}") for i in range(2)]

            with tc.tile_pool(name="pp" + sfx, bufs=3, space="PSUM") as pp, \
                 tc.tile_pool(name="ppv" + sfx, bufs=2, space="PSUM") as ppv, \
                 tc.tile_pool(name="ppb" + sfx, bufs=1, space="PSUM") as ppb:

                def rope(ps, dst, win, nm):
                    """dst[:, win] = rope(ps) via rotate-half copies + ct/st."""
                    rot = scr.tile([128, W5], F32, tag="rot", name=f"rot{nm}")
                    for hb in range(2):
                        nc.scalar.copy(rot[ds(hb * 64, 32), :], ps[ds(hb * 64 + 32, 32), :])
                        nc.scalar.copy(rot[ds(hb * 64 + 32, 32), :], ps[ds(hb * 64, 32), :])
                    nc.vector.tensor_tensor(rot[:], rot[:], st_sb[:, win], MUL)
                    nc.vector.tensor_tensor(dst[:, win], ps[:], ct_sb[:, win], MUL)
                    nc.vector.tensor_tensor(dst[:, win], dst[:, win], rot[:], ADD)

                for w in range(NW):
                    win = ds(w * W5, W5)
                    xhw, xlw = x0 if w == 0 else load_x(w, f"_{w}{sfx}")
                    # k: 3-chain fp16 hi/lo split, fp32 PSUM accumulation
                    for mt in range(2):
                        ps = pp.tile([128, W5], F32, tag="proj")
                        chains = ((wkh_sb, xhw), (wkh_sb, xlw), (wkl_sb, xhw))
                        n = len(chains) * KT
                        i = 0
                        for wt_, xt_ in chains:
                            for kt_i in range(KT):
                                nc.tensor.matmul(ps[:], wt_[:, kt_i, ds(mt * 128, 128)],
                                                 xt_[:, kt_i, :],
                                                 start=(i == 0), stop=(i == n - 1))
                                i += 1
                        rope(ps, kT[mt], win, f"k{mt}_{w}{sfx}")
                    # q: single hi chain
                    for mt in range(2):
                        psq = pp.tile([128, W5], F32, tag="proj")
                        for kt_i in range(KT):
                            nc.tensor.matmul(psq[:], wq_sb[:, kt_i, ds(mt * 128, 128)],
                                             xhw[:, kt_i, :],
                                             start=(kt_i == 0), stop=(kt_i == KT - 1))
                        rope(psq, qT[mt], win, f"q{mt}_{w}{sfx}")
                    # v: 3-chain split (x chunks stationary); beta: hi chain
                    for sub in range(W5 // C):
                        tt = w * (W5 // C) + sub
                        cs = ds(sub * C, C)
                        psv = ppv.tile([128, GD], F32, tag="pv")
                        chains = ((xhw, wvh_sb), (xlw, wvh_sb), (xhw, wvl_sb))
                        n = len(chains) * KT
                        i = 0
                        for xt_, wt_ in chains:
                            for kt_i in range(KT):
                                nc.tensor.matmul(psv[:], xt_[:, kt_i, cs], wt_[:, kt_i, :],
                                                 start=(i == 0), stop=(i == n - 1))
                                i += 1
                        psb = ppb.tile([128, HG], F32, tag="pb")
                        for kt_i in range(KT):
                            nc.tensor.matmul(psb[:], xhw[:, kt_i, cs], wb_sb[:, kt_i, :],
                                             start=(kt_i == 0), stop=(kt_i == KT - 1))
                        nc.scalar.copy(v_sb[:, tt, :], psv[:])
                        nc.vector.tensor_tensor(blog[:, tt, :], psb[:], bbx_sb[:], ADD)
                nc.scalar.activation(beta[:], blog[:], mybir.ActivationFunctionType.Sigmoid)

                # ---- |k|, |v| sums -> global threshold ----
                acc = tiny.tile([128, 4], F32, tag="acc", name="acc" + sfx)
                nc.vector.tensor_reduce(acc[:, 0:1], kT[0][:], axis=mybir.AxisListType.X,
                                        op=ADD, apply_absolute_value=True)
                nc.vector.tensor_reduce(acc[:, 1:2], kT[1][:], axis=mybir.AxisListType.X,
                                        op=ADD, apply_absolute_value=True)
                nc.vector.tensor_reduce(acc[:, 2:3], v_sb[:], axis=mybir.AxisListType.XY,
                                        op=ADD, apply_absolute_value=True)
                nc.vector.memset(acc[:, 3:4], 0.0)
                ones = tiny.tile([128, 1], F32, tag="ones", name="ones" + sfx)
                nc.vector.memset(ones[:], 1.0)
                pst = ppb.tile([1, 4], F32, tag="pt")
                nc.tensor.matmul(pst[:], ones[:], acc[:], start=True, stop=True)
                sc4 = tiny.tile([1, 4], F32, tag="sc4", name="sc4" + sfx)
                nc.vector.tensor_copy(sc4[:], pst[:])
                sc = tiny.tile([1, 2], F32, tag="sc", name="sc" + sfx)
                nc.vector.tensor_tensor(sc[0:1, 0:1], sc4[0:1, 0:1], sc4[0:1, 1:2], ADD)
                nc.vector.tensor_copy(sc[0:1, 1:2], sc4[0:1, 2:3])
                if use_cc:
                    nc.sync.dma_start(cc_in[:], sc[:])
                    nc.gpsimd.collective_compute(
                        "AllReduce", ADD,
                        replica_groups=[list(range(n_cores))],
                        ins=[cc_in[:]], outs=[cc_out[:]])
                    tot = tiny.tile([1, 2], F32, tag="tot", name="tot" + sfx)
                    nc.sync.dma_start(tot[:], cc_out[:])
                else:
                    tot = sc
                n_elem = float(B * T * INNER) if use_cc else float(T * GD)
                thr1 = tiny.tile([1, 4], F32, tag="thr1", name="thr1" + sfx)
                nc.vector.tensor_scalar(thr1[0:1, 0:2], tot[0:1, :], 1.0 / n_elem, None, MUL)
                nc.vector.tensor_scalar(thr1[0:1, 0:2], thr1[0:1, 0:2], THR_MIN, THR_MAX,
                                        mybir.AluOpType.max, mybir.AluOpType.min)
                nc.vector.tensor_scalar(thr1[0:1, 2:4], thr1[0:1, 0:2], -1.0, None, MUL)
                ones1 = tiny.tile([1, 128], F32, tag="ones1", name="ones1" + sfx)
                nc.vector.memset(ones1[:], 1.0)
                psth = ppb.tile([128, 4], F32, tag="pth")
                nc.tensor.matmul(psth[:], ones1[:], thr1[:], start=True, stop=True)
                thrb = tiny.tile([128, 4], F32, tag="thrb", name="thrb" + sfx)
                nc.vector.tensor_copy(thrb[:], psth[:])   # [thr_k, thr_v, -thr_k, -thr_v]

                # ---- ternarize k -> fp16 ----
                for mt in range(2):
                    for w in range(NW):
                        win = ds(w * W5, W5)
                        g = gl.tile([128, W5], F32, tag="g")
                        l = gl.tile([128, W5], F32, tag="l")
                        nc.gpsimd.tensor_scalar(g[:], kT[mt][:, win], thrb[:, 0:1], None,
                                                mybir.AluOpType.is_gt)
                        nc.gpsimd.tensor_scalar(l[:], kT[mt][:, win], thrb[:, 2:3], None,
                                                mybir.AluOpType.is_lt)
                        nc.vector.tensor_tensor(kTt[mt][:, win], g[:], l[:], SUB)

                # ---- ternarize v + fold beta -> fp16 ----
                nwc = W5 // GD
                for w in range(NCH // nwc):
                    winc = ds(w * nwc, nwc)
                    vin = v_sb[:, winc, :]
                    g = gl.tile([128, W5], F32, tag="g")
                    l = gl.tile([128, W5], F32, tag="l")
                    gv = g[:].rearrange("p (c m) -> p c m", m=GD)
                    lv = l[:].rearrange("p (c m) -> p c m", m=GD)
                    nc.gpsimd.tensor_scalar(gv, vin, thrb[:, 1:2], None, mybir.AluOpType.is_gt)
                    nc.gpsimd.tensor_scalar(lv, vin, thrb[:, 3:4], None, mybir.AluOpType.is_lt)
                    nc.vector.tensor_tensor(gv, gv, lv, SUB)
                    bcast = beta[:, winc, :, None].to_broadcast([128, nwc, HG, HD])
                    nc.vector.tensor_tensor(
                        vbt[:, winc, :].rearrange("p c (h d) -> p c h d", d=HD),
                        g[:].rearrange("p (c h d) -> p c h d", h=HG, d=HD), bcast, MUL)

            vB = vbt[:]
            wo_sb = wpool.tile([128, 2, D_MODEL], F16, tag="wslot", name="wo" + sfx)
            nc.sync.dma_start(wo_sb[:], wo_d.rearrange("(t p) m -> p t m", p=128))

            # ---- banded attention with inlined output projection ----
            # PSUM rule: matmuls sharing a bank must share a row-group, so
            # heads are grouped by partition offset: slot order [h0, h2, h1, h3]
            # (dt2 is host-reordered to match).
            with tc.tile_pool(name="ppa" + sfx, bufs=2, space="PSUM") as ppa:
                av_stage = [None, None]
                for jc in range(NCH):
                    ilen = min(2 * C, T - jc * C)
                    sts = stp.tile([128, HG, 2 * C], F16, tag="sts")
                    for grp in range(2):
                        spg = ppa.tile([128, 2, 2 * C], F32, tag=f"s{grp}",
                                       name=f"s{grp}_{jc}")
                        for j, h in enumerate((grp, grp + 2)):
                            tl, po = h // 2, (h % 2) * 64
                            nc.tensor.matmul(
                                spg[:, j, 0:ilen],
                                kTt[tl][ds(po, 64), ds(jc * C, C)],
                                qT[tl][ds(po, 64), ds(jc * C, ilen)],
                                start=True, stop=True)
                        nc.vector.tensor_tensor(sts[:, ds(grp * 2, 2), 0:ilen],
                                                spg[:, :, 0:ilen],
                                                dt2_sb[:, ds(grp * 2, 2), 0:ilen], MUL)
                    avs = []
                    for hp in range(2):
                        av = ppa.tile([128, 2, 2 * C], F32, tag=f"av{hp}")
                        for hh in range(2):
                            h = hp * 2 + hh
                            slot = (h % 2) * 2 + h // 2
                            nc.tensor.matmul(av[0:64, hh, 0:ilen],
                                             vB[:, jc, ds(h * HD, HD)],
                                             sts[:, slot, 0:ilen],
                                             start=True, stop=True)
                        avs.append(av)
                    for hp in range(2):
                        for hh in range(2):
                            dst = oT[hp][ds(hh * 64, 64), ds(jc * C, C)]
                            if jc == 0:
                                nc.vector.tensor_copy(dst, avs[hp][0:64, hh, 0:C])
                            else:
                                nc.vector.tensor_tensor(dst, avs[hp][0:64, hh, 0:C],
                                                        av_stage[hp][0:64, hh, :], ADD)
                    if jc < NCH - 1:
                        stage = []
                        for hp in range(2):
                            s_ = stp.tile([64, 2, C], F32, tag=f"avst{hp}", name=f"avst{hp}_{jc}")
                            nc.scalar.copy(s_[:], avs[hp][0:64, :, C:2 * C])
                            stage.append(s_)
                        av_stage = stage
                    # output projection for the chunk just completed
                    for nn in range(D_MODEL // 512):
                        pf = ppa.tile([128, 512], F32, tag=f"s{nn}", name=f"pf{jc}_{nn}")
                        for hp in range(2):
                            nc.tensor.matmul(pf[:], oT[hp][:, ds(jc * 128, 128)],
                                             wo_sb[:, hp, ds(nn * 512, 512)],
                                             start=(hp == 0), stop=(hp == 1))
                        fo = gl.tile([128, 512], F32, tag="g", name=f"fo{jc}_{nn}")
                        nc.scalar.copy(fo[:], pf[:])
                        nc.sync.dma_start(out_d[ds(jc * 128, 128), ds(nn * 512, 512)], fo[:])

    nc.finalize()
    return nc


def _host_tables(T, alpha_log, heads):
    inv = (np.float32(1.0) /
           (np.float32(ROPE_BASE) ** (np.arange(0, HD, 2, dtype=np.float32) / np.float32(HD))))
    tpos = np.arange(T, dtype=np.float32)
    freqs = tpos[None, :] * inv[:, None]          # [32, T] fp32
    cos32 = np.cos(freqs).astype(np.float32)
    sin32 = np.sin(freqs).astype(np.float32)
    ct = np.empty((128, T), np.float32)
    st = np.empty((128, T), np.float32)
    for r in range(128):
        jj = r % HD
        idx = jj % 32
        ct[r] = cos32[idx]
        st[r] = (-sin32[idx]) if jj < 32 else sin32[idx]

    alpha = (1.0 / (1.0 + np.exp(-alpha_log.astype(np.float32)))).astype(np.float32)
    la = np.log(np.clip(alpha[:, 0], np.float32(1e-6), None)).astype(np.float32)
    need = 104.0 / np.abs(la).min()
    assert need <= 2 * C, f"decay band too wide for 2-chunk attention: {need}"
    jl = np.arange(C, dtype=np.float32)
    il = np.arange(2 * C, dtype=np.float32)
    diff = (il[None, :] - jl[:, None]).astype(np.float32)   # [128, 256]
    dt2 = np.zeros((128, HG, 2 * C), np.float32)
    slot_order = (0, 2, 1, 3)   # psum row-group pairing; see build_nc
    with np.errstate(over="ignore"):
        for s_, hh in enumerate(slot_order):
            m = np.exp(diff * la[heads[hh]], dtype=np.float32)
            m[diff < 0] = 0.0
            dt2[:, s_, :] = m
    return ct, st, dt2


def _split16(a):
    hi = a.astype(np.float16)
    lo = (a.astype(np.float32) - hi.astype(np.float32)).astype(np.float16)
    return np.ascontiguousarray(hi), np.ascontiguousarray(lo)


def make_in_maps(x, Wq, Wk, Wv, Wo, Wb, bb, alpha_log, T):
    maps = []
    for c in range(N_CORES):
        b, hg = c // 4, c % 4
        sl = slice(hg * GD, (hg + 1) * GD)
        heads = list(range(hg * HG, (hg + 1) * HG))
        xh, xl = _split16(np.ascontiguousarray(x[b].T).astype(np.float32))
        wkh, wkl = _split16(np.ascontiguousarray(Wk[sl].T))
        wvh, wvl = _split16(np.ascontiguousarray(Wv[sl].T))
        ct, st, dt2 = _host_tables(T, alpha_log, heads)
        maps.append({
            "xh": xh, "xl": xl,
            "wkh": wkh, "wkl": wkl,
            "wvh": wvh, "wvl": wvl,
            "wq": np.ascontiguousarray(Wq[sl].T).astype(np.float16),
            "wb": np.ascontiguousarray(Wb[heads].T).astype(np.float16),
            "bbx": np.tile(bb[heads][None, :], (128, 1)).astype(np.float32),
            "wo": np.ascontiguousarray(Wo[:, sl].T).astype(np.float16),
            "ct": ct, "st": st, "dt2": dt2,
        })
    return maps


def kernel(x, Wq, Wk, Wv, Wo, Wb, bb, alpha_log):
    x = np.asarray(x, dtype=np.float32)
    T = x.shape[1]
    key = (T, N_CORES, True, 1)
    if key not in _NC_CACHE:
        _NC_CACHE[key] = build_nc(T=T, n_cores=N_CORES, use_cc=True, repeat=1)
    nc = _NC_CACHE[key]
    maps = make_in_maps(x, np.asarray(Wq, np.float32), np.asarray(Wk, np.float32),
                        np.asarray(Wv, np.float32), np.asarray(Wo, np.float32),
                        np.asarray(Wb, np.float32), np.asarray(bb, np.float32),
                        np.asarray(alpha_log, np.float32), T)
    res = run_bass_kernel_spmd(nc, maps, list(range(N_CORES)))
    out = np.zeros((B, T, D_MODEL), np.float32)
    for c in range(N_CORES):
        out[c // 4] += res.results[c]["out"]
    return out
